# revision 1
# baseline (speedup 1.0000x reference)
"""nn_BaselineClassifier GNN message-passing kernel for 8 trn2 NeuronCores.

Distribution (per the sharding hint):
  - Edges sharded across the 8 cores on the E axis (1.6M / 8 = 200k per core),
    each shard sorted by destination node on the host (sharding prep).
  - MLP weights, embedding tables and node state x replicated on every core.
  - Per-node partial segment sums all-reduced (psum) after each layer.

Math restructure (exact, saves two full msg passes):
  msg = MLP(ea) is layer-independent, so segment_sum(x[src] + msg) =
  segment_sum(x[src]) + S with S = segment_sum(msg) computed once. Each layer:
      x <- (segsum(x[src]) + x + S) / deg.

Segment sums avoid XLA scatter (which is unstable/slow on this backend, and
the Ant Bass dma_scatter_add instruction was shown by HW probes to lose
updates on duplicate destination indices): with dst-sorted edge shards,
segment_sum = diff of an exclusive cumsum gathered at per-node boundary
offsets. Pooling uses static per-graph slices (batch is sorted).
"""
import numpy as np

N_NODES = 100_000
N_EDGES = 1_600_000
NCORES = 8
E_SH = N_EDGES // NCORES
NUM_GRAPHS = 64
LAYERS = 3

_cache = {}


def _seg_sum(v, bnd):
    """segment sum of v [E,w] whose rows are dst-sorted; bnd [N+1] boundaries."""
    import jax.numpy as jnp
    P = jnp.concatenate([jnp.zeros((1, v.shape[1]), v.dtype),
                         jnp.cumsum(v, axis=0)], axis=0)
    at = P[bnd]                       # [N+1, w]
    return at[1:] - at[:-1]           # [N, w]


def _build(gb):
    if "fn" in _cache:
        return _cache["fn"]
    import jax
    import jax.numpy as jnp
    from jax.sharding import Mesh, PartitionSpec as P
    try:
        from jax.experimental.shard_map import shard_map
    except ImportError:
        from jax import shard_map

    devs = jax.devices()[:NCORES]
    mesh = Mesh(np.asarray(devs), ("c",))

    def body(row, ports, flags, eattr, bnd,
             emb_port, emb_flags, W1, b1, W2, b2, CW1, Cb1, CW2, Cb2):
        row = row.reshape(-1)
        ports = ports.reshape(-1)
        flags = flags.reshape(-1)
        eattr = eattr.reshape(-1, eattr.shape[-1])
        bnd = bnd.reshape(-1)

        ea = jnp.concatenate([eattr, emb_port[ports], emb_flags[flags]], axis=1)
        cnt = jax.lax.psum((bnd[1:] - bnd[:-1]).astype(jnp.float32), "c")
        sum_ea = jax.lax.psum(_seg_sum(ea, bnd), "c")
        loop_attr = sum_ea / jnp.maximum(cnt, 1.0)[:, None]

        def mlp(v):
            return jnp.maximum(v @ W1 + b1, 0.0) @ W2 + b2

        msg_e = mlp(ea)                       # [E_SH, 64]
        msg_self = mlp(loop_attr)             # [N, 64] replicated compute
        S = jax.lax.psum(_seg_sum(msg_e, bnd), "c") + msg_self
        deg = (cnt + 1.0)[:, None]

        x = S / deg                           # layer 1 (x0 = 0)
        for _ in range(LAYERS - 1):
            t = jax.lax.psum(_seg_sum(x[row], bnd), "c")
            x = (t + x + S) / deg

        # pooling: batch sorted -> static per-graph slices
        means, maxs = [], []
        for g in range(NUM_GRAPHS):
            a, b = int(gb[g]), int(gb[g + 1])
            if b > a:
                seg = x[a:b]
                means.append(seg.mean(axis=0))
                maxs.append(seg.max(axis=0))
            else:
                means.append(jnp.zeros((x.shape[1],), x.dtype))
                maxs.append(jnp.full((x.shape[1],), -jnp.inf, x.dtype))
        pooled = jnp.concatenate(
            [jnp.stack(means), jnp.stack(maxs)], axis=1)  # [64, 128]
        out = jnp.maximum(pooled @ CW1 + Cb1, 0.0) @ CW2 + Cb2
        return out[None]

    sharded, repl = P("c"), P()
    in_specs = (sharded,) * 5 + (repl,) * 10
    fn = jax.jit(
        shard_map(body, mesh=mesh, in_specs=in_specs, out_specs=P("c"),
                  check_rep=False))
    _cache["fn"] = fn
    return fn


def kernel(edge_index, dst_ports, tcp_flags, edge_attr, batch,
           emb_port, emb_flags, W1, b1, W2, b2, CW1, Cb1, CW2, Cb2):
    i32 = lambda a: np.asarray(a, np.int32)
    f32 = lambda a: np.asarray(a, np.float32)

    row_all = i32(edge_index[0])
    col_all = i32(edge_index[1])
    ports_all = i32(dst_ports)
    flags_all = i32(tcp_flags)
    eattr_all = f32(edge_attr)
    batch_np = i32(batch)

    # shard edges on E; sort each shard by dst (sharding prep)
    row = np.empty((NCORES, E_SH), np.int32)
    ports = np.empty((NCORES, E_SH), np.int32)
    flags = np.empty((NCORES, E_SH), np.int32)
    eattr = np.empty((NCORES, E_SH, eattr_all.shape[1]), np.float32)
    bnd = np.empty((NCORES, N_NODES + 1), np.int32)
    for c in range(NCORES):
        sl = slice(c * E_SH, (c + 1) * E_SH)
        cs = col_all[sl]
        o = np.argsort(cs, kind="stable")
        row[c] = row_all[sl][o]
        ports[c] = ports_all[sl][o]
        flags[c] = flags_all[sl][o]
        eattr[c] = eattr_all[sl][o]
        bnd[c] = np.searchsorted(cs[o], np.arange(N_NODES + 1))

    gb = np.searchsorted(batch_np, np.arange(NUM_GRAPHS + 1))
    fn = _build(gb)
    out = fn(row, ports, flags, eattr, bnd,
             f32(emb_port), f32(emb_flags), f32(W1), f32(b1), f32(W2), f32(b2),
             f32(CW1), f32(Cb1), f32(CW2), f32(Cb2))
    return np.asarray(out)[0]

